# revision 12
# baseline (speedup 1.0000x reference)
"""MoE layer (16 experts, top-2, SwiGLU FFN + shared expert) on 8 Trainium2 cores.

Strategy (expert-parallel, host-side dispatch):
  - Host computes the gate (logits -> softmax -> top-2 -> combine weights) in
    float64 numpy; the min 2nd-vs-3rd logit gap at these scales is ~1e-4, so
    the top-2 set matches the f32 reference selection robustly.
  - Core m owns experts 2m and 2m+1.  Tokens routed to an expert are gathered,
    transposed to feature-major [D, T] and padded to a fixed capacity C.
  - Each core also runs the shared expert on a 1/8 slice of all tokens.
  - The device kernel does three dense SwiGLU blocks (expert A, expert B,
    shared slice) in feature-major layout: all matmuls keep weights stationary
    ([K=128 d, M=128 f] tiles) and stream token columns, so no on-chip
    transposes are needed.  Matmuls run as float32r (full PE rate at N>=256).
  - Host applies the combine weights and scatter-adds expert contributions in
    expert-id order (matching the reference's accumulation order), then adds
    the shared-expert output.
"""

import numpy as np

E, TOPK, D, F = 16, 2, 2048, 1408
B, S = 4, 2048
N = B * S
NCORES = 8
C = 1024               # per-expert token capacity (multiple of 512 keeps all
                       # matmul chunks at the fp32r-friendly N=512; the rare
                       # overflow tokens are finished on the host)
NS = N // NCORES       # shared-expert tokens per core
TBLOCKS = [(0, C), (C, C), (2 * C, NS)]   # (col offset, ncols) per swiglu block
TCOLS = 2 * C + NS
KD = D // 128          # 16 k-tiles over D
MF = F // 128          # 11 m-tiles over F

_prog_cache = {}


def _emit(ctx, tc, xT, gup, wdp, yT):
    import concourse.bass as bass  # noqa: F401
    from concourse import mybir

    nc = tc.nc
    f32 = mybir.dt.float32
    f32r = mybir.dt.float32r
    silu = mybir.ActivationFunctionType.Silu

    xpool = ctx.enter_context(tc.tile_pool(name="x", bufs=1))
    hpool = ctx.enter_context(tc.tile_pool(name="h", bufs=2))
    wpool = ctx.enter_context(tc.tile_pool(name="w", bufs=2))
    opool = ctx.enter_context(tc.tile_pool(name="o", bufs=3))
    tpool = ctx.enter_context(tc.tile_pool(name="t", bufs=2))
    pspool = ctx.enter_context(tc.tile_pool(name="ps", bufs=1, space="PSUM"))

    # PE warm-up: ~18us of dummy matmuls so the HAM clock-gate releases
    # (1.2->2.4 GHz) while the first x/weight DMAs are still in flight.
    zsrc32 = wpool.tile([128, 512], f32, tag="warm32", bufs=1, name="zsrc32")
    nc.vector.memset(zsrc32[:], 0.0)
    zsrc = wpool.tile([128, 512], f32r, tag="warm", bufs=1, name="zsrc")
    nc.vector.tensor_copy(zsrc[:], zsrc32[:])
    pwarm = pspool.tile([128, 512], f32, tag="d0", bufs=2, name="pwarm")
    for i in range(56):
        nc.tensor.matmul(pwarm[:, :], zsrc[:, 0:128], zsrc[:, :],
                         start=(i == 0), stop=(i == 55))

    for b, (off, T) in enumerate(TBLOCKS):
        nch = T // 512                      # all chunks are exactly 512 wide
        chunks = [c0 * 512 for c0 in range(nch)]

        x_sb = xpool.tile([128, KD, T], f32r, tag="x", name=f"x{b}")
        w0 = {}
        if b == 0:
            # prefetch the m=0 weight tiles in engine-spread pieces before the
            # (larger) x loads so the first matmul isn't gated on one transfer
            for half in range(2):
                w_sb = wpool.tile([128, (KD // 2) * 256], f32r, tag="w",
                                  name=f"w{b}_0_{half}")
                wsl = gup[b, 0, :, half * (KD // 2) * 256:(half + 1) * (KD // 2) * 256]
                for j in range(KD // 2):
                    nc.sync.dma_start(w_sb[:, j * 256:(j + 1) * 256],
                                      wsl[:, j * 256:(j + 1) * 256])
                w0[half] = w_sb
            for ci in range(nch):
                nc.sync.dma_start(x_sb[:, 0, ci * 512:(ci + 1) * 512],
                                  xT[0:128, off + ci * 512:off + ci * 512 + 512])
            for k in range(1, KD):
                nc.sync.dma_start(x_sb[:, k, :], xT[k * 128:(k + 1) * 128, off:off + T])
        else:
            for k in range(KD):
                nc.sync.dma_start(x_sb[:, k, :], xT[k * 128:(k + 1) * 128, off:off + T])

        h_sb = hpool.tile([128, MF, T], f32r, tag="h", name=f"h{b}")

        # phase 1: gate/up matmuls + silu*up -> h
        for m in range(MF):
            pg = [pspool.tile([128, 512], f32, tag=f"g{ci}", name=f"pg{b}_{m}_{ci}")
                  for ci in range(nch)]
            pu = [pspool.tile([128, 512], f32, tag=f"u{ci}", name=f"pu{b}_{m}_{ci}")
                  for ci in range(nch)]
            for half in range(2):
                if b == 0 and m == 0:
                    w_sb = w0[half]
                else:
                    w_sb = wpool.tile([128, (KD // 2) * 256], f32r, tag="w",
                                      name=f"w{b}_{m}_{half}")
                    wsl = gup[b, m, :, half * (KD // 2) * 256:(half + 1) * (KD // 2) * 256]
                    hw = (KD // 2) * 128  # split across two DMA engines
                    nc.sync.dma_start(w_sb[:, :hw], wsl[:, :hw])
                    nc.sync.dma_start(w_sb[:, hw:], wsl[:, hw:])
                for j in range(KD // 2):
                    k = half * (KD // 2) + j
                    lg = w_sb[:, (2 * j) * 128:(2 * j + 1) * 128]
                    lu = w_sb[:, (2 * j + 1) * 128:(2 * j + 2) * 128]
                    for ci, c0 in enumerate(chunks):
                        nc.tensor.matmul(pg[ci][:, :], lg,
                                         x_sb[:, k, c0:c0 + 512],
                                         start=(k == 0), stop=(k == KD - 1))
                    for ci, c0 in enumerate(chunks):
                        nc.tensor.matmul(pu[ci][:, :], lu,
                                         x_sb[:, k, c0:c0 + 512],
                                         start=(k == 0), stop=(k == KD - 1))
            for ci, c0 in enumerate(chunks):
                sil = tpool.tile([128, 512], f32, tag="t", name=f"s{b}_{m}_{ci}")
                nc.scalar.activation(sil[:], pg[ci][:], silu)
                nc.vector.tensor_mul(h_sb[:, m, c0:c0 + 512], sil[:], pu[ci][:])

        # phase 2: down matmul -> yT
        for m2 in range(KD):
            wd_sb = wpool.tile([128, F], f32r, tag="wd", name=f"wd{b}_{m2}")
            nc.sync.dma_start(wd_sb[:, :F // 2], wdp[b, m2, :, :F // 2])
            nc.sync.dma_start(wd_sb[:, F // 2:], wdp[b, m2, :, F // 2:])
            pd = [pspool.tile([128, 512], f32, tag=f"d{ci}", bufs=2,
                              name=f"pd{b}_{m2}_{ci}") for ci in range(nch)]
            for kf in range(MF):
                ld = wd_sb[:, kf * 128:(kf + 1) * 128]
                for ci, c0 in enumerate(chunks):
                    nc.tensor.matmul(pd[ci][:, :], ld,
                                     h_sb[:, kf, c0:c0 + 512],
                                     start=(kf == 0), stop=(kf == MF - 1))
            for ci, c0 in enumerate(chunks):
                o_sb = opool.tile([128, 512], f32, tag="o", name=f"o{b}_{m2}_{ci}")
                nc.vector.tensor_copy(o_sb[:], pd[ci][:])
                nc.sync.dma_start(
                    yT[m2 * 128:(m2 + 1) * 128, off + c0:off + c0 + 512], o_sb[:])


def _build_program():
    from contextlib import ExitStack

    import concourse.tile as tile
    from concourse import bacc, mybir

    nc = bacc.Bacc("TRN2", target_bir_lowering=False, debug=False,
                   enable_asserts=False, num_devices=NCORES)
    f32 = mybir.dt.float32
    xT = nc.dram_tensor("xT", [D, TCOLS], mybir.dt.float32r, kind="ExternalInput").ap()
    gup = nc.dram_tensor("gup", [3, MF, 128, KD * 256], mybir.dt.float32r, kind="ExternalInput").ap()
    wdp = nc.dram_tensor("wdp", [3, KD, 128, F], mybir.dt.float32r, kind="ExternalInput").ap()
    yT = nc.dram_tensor("yT", [D, TCOLS], f32, kind="ExternalOutput").ap()

    with tile.TileContext(nc) as tc, ExitStack() as ctx:
        _emit(ctx, tc, xT, gup, wdp, yT)
    nc.compile()
    return nc


def _get_program():
    if "nc" not in _prog_cache:
        _prog_cache["nc"] = _build_program()
    return _prog_cache["nc"]


def _pack_gu(wg, wu):
    # [F, D] x2 -> [MF, 128, KD*2*128]; tile [:, :, (k*2+g)*128 + f]
    g = wg.reshape(MF, 128, KD, 128).transpose(0, 3, 2, 1)   # [m, p, k, f]
    u = wu.reshape(MF, 128, KD, 128).transpose(0, 3, 2, 1)
    return np.ascontiguousarray(
        np.stack([g, u], axis=3).reshape(MF, 128, KD * 256))


def _pack_wd(wd):
    # [D, F] -> [KD, 128, F]; tile [:, :, kf*128 + j] = wd[m2*128+j, kf*128+p]
    return np.ascontiguousarray(
        wd.reshape(KD, 128, MF, 128).transpose(0, 3, 2, 1).reshape(KD, 128, F))


def _swiglu_np(x, wg, wu, wd):
    # numpy fallback for capacity overflow (float32, matches reference math)
    a = x @ wg.T
    h = (a / (1.0 + np.exp(-a))) * (x @ wu.T)
    return h @ wd.T


def _ensure_axon_hooks():
    """Make ``antenv.axon_hooks`` importable (bass_utils needs it when
    BASS_TRACE=1 under axon; some images ship antenv without it)."""
    try:
        import antenv.axon_hooks  # noqa: F401
        return
    except ImportError:
        pass
    import sys
    import types

    mod = types.ModuleType("antenv.axon_hooks")
    mod._hook = None

    def set_axon_ntff_profile_hook(h):
        mod._hook = h

    def get_axon_ntff_profile_hook():
        return mod._hook

    mod.set_axon_ntff_profile_hook = set_axon_ntff_profile_hook
    mod.get_axon_ntff_profile_hook = get_axon_ntff_profile_hook
    try:
        import antenv

        sys.modules["antenv.axon_hooks"] = mod
        antenv.axon_hooks = mod
    except ImportError:
        return
    try:
        from trn_agent_boot.trn_boot import _ntff_profile_via_ctypes

        mod._hook = _ntff_profile_via_ctypes("/opt/axon/libaxon_pjrt.so")
    except Exception:
        pass


def kernel(x, gate_w, w_gate, w_up, w_down, sw_gate, sw_up, sw_down, expert_bias):
    from concourse.bass_utils import run_bass_kernel_spmd

    _ensure_axon_hooks()

    x = np.asarray(x, np.float32)
    gate_w = np.asarray(gate_w, np.float32)
    w_gate = np.asarray(w_gate, np.float32)
    w_up = np.asarray(w_up, np.float32)
    w_down = np.asarray(w_down, np.float32)
    sw_gate = np.asarray(sw_gate, np.float32)
    sw_up = np.asarray(sw_up, np.float32)
    sw_down = np.asarray(sw_down, np.float32)
    expert_bias = np.asarray(expert_bias, np.float32)

    flat = x.reshape(N, D)

    # ---- host gating / routing ----
    logits = flat.astype(np.float64) @ gate_w.astype(np.float64).T
    biased = logits + expert_bias.astype(np.float64)[None, :]
    order = np.argsort(-biased, axis=1, kind="stable")
    top_idx = order[:, :TOPK]                                  # [N, 2]
    m64 = logits.max(axis=1, keepdims=True)
    p = np.exp(logits - m64)
    probs = p / p.sum(axis=1, keepdims=True)
    top_w = np.take_along_axis(probs, top_idx, axis=1)
    top_w = top_w / top_w.sum(axis=1, keepdims=True)           # [N, 2]

    idx_list, w_list = [], []
    for e in range(E):
        sel = (top_idx == e)
        rows = np.where(sel.any(axis=1))[0]
        we = np.where(sel[rows, 0], top_w[rows, 0], top_w[rows, 1]).astype(np.float32)
        idx_list.append(rows)
        w_list.append(we)

    # ---- build per-core inputs ----
    sw_gu = _pack_gu(sw_gate, sw_up)
    sw_d = _pack_wd(sw_down)
    in_maps = []
    for c in range(NCORES):
        xT = np.zeros((D, TCOLS), np.float32)
        gu = np.empty((3, MF, 128, KD * 256), np.float32)
        wd = np.empty((3, KD, 128, F), np.float32)
        for half in range(2):
            e = 2 * c + half
            rows = idx_list[e][:C]
            xT[:, half * C:half * C + len(rows)] = flat[rows].T
            gu[half] = _pack_gu(w_gate[e], w_up[e])
            wd[half] = _pack_wd(w_down[e])
        xT[:, 2 * C:] = flat[c * NS:(c + 1) * NS].T
        gu[2] = sw_gu
        wd[2] = sw_d
        in_maps.append({"xT": xT, "gup": gu, "wdp": wd})

    # ---- run on 8 cores ----
    nc = _get_program()
    res = run_bass_kernel_spmd(nc, in_maps, core_ids=list(range(NCORES)))
    _prog_cache["last_results"] = res

    # ---- combine (expert-id order, then shared — matches reference) ----
    out = np.zeros((N, D), np.float32)
    for e in range(E):
        c, half = divmod(e, 2)
        rows, we = idx_list[e], w_list[e]
        ndev = min(len(rows), C)
        y = res.results[c]["yT"][:, half * C:half * C + ndev].T   # [ndev, D]
        out[rows[:ndev]] += we[:ndev, None] * y
        if len(rows) > C:  # capacity overflow: finish the tail on host
            r2 = rows[C:]
            y2 = _swiglu_np(flat[r2], w_gate[e], w_up[e], w_down[e])
            out[r2] += we[C:, None] * y2
    for c in range(NCORES):
        out[c * NS:(c + 1) * NS] += res.results[c]["yT"][:, 2 * C:].T

    return out.reshape(B, S, D)


# revision 14
# speedup vs baseline: 1.0265x; 1.0265x over previous
"""MoE layer (16 experts, top-2, SwiGLU FFN + shared expert) on 8 Trainium2 cores.

Strategy (expert-parallel, host-side dispatch):
  - Host computes the gate (logits -> softmax -> top-2 -> combine weights) in
    float64 numpy; the min 2nd-vs-3rd logit gap at these scales is ~1e-4, so
    the top-2 set matches the f32 reference selection robustly.
  - Core m owns experts 2m and 2m+1.  Tokens routed to an expert are gathered,
    transposed to feature-major [D, T] and padded to a fixed capacity C.
  - Each core also runs the shared expert on a 1/8 slice of all tokens.
  - The device kernel does three dense SwiGLU blocks (expert A, expert B,
    shared slice) in feature-major layout: all matmuls keep weights stationary
    ([K=128 d, M=128 f] tiles) and stream token columns, so no on-chip
    transposes are needed.  Matmuls run as float32r (full PE rate at N>=256).
  - Host applies the combine weights and scatter-adds expert contributions in
    expert-id order (matching the reference's accumulation order), then adds
    the shared-expert output.
"""

import numpy as np

E, TOPK, D, F = 16, 2, 2048, 1408
B, S = 4, 2048
N = B * S
NCORES = 8
C = 1024               # per-expert token capacity (multiple of 512 keeps all
                       # matmul chunks at the fp32r-friendly N=512; the rare
                       # overflow tokens are finished on the host)
NS = N // NCORES       # shared-expert tokens per core
TBLOCKS = [(0, C), (C, C), (2 * C, NS)]   # (col offset, ncols) per swiglu block
TCOLS = 2 * C + NS
KD = D // 128          # 16 k-tiles over D
MF = F // 128          # 11 m-tiles over F

_prog_cache = {}


def _emit(ctx, tc, xT, gup, wdp, yT):
    import concourse.bass as bass  # noqa: F401
    from concourse import mybir

    nc = tc.nc
    f32 = mybir.dt.float32
    f32r = mybir.dt.float32r
    silu = mybir.ActivationFunctionType.Silu

    xpool = ctx.enter_context(tc.tile_pool(name="x", bufs=1))
    hpool = ctx.enter_context(tc.tile_pool(name="h", bufs=2))
    wpool = ctx.enter_context(tc.tile_pool(name="w", bufs=2))
    opool = ctx.enter_context(tc.tile_pool(name="o", bufs=4))
    tpool = ctx.enter_context(tc.tile_pool(name="t", bufs=2))
    pspool = ctx.enter_context(tc.tile_pool(name="ps", bufs=1, space="PSUM"))

    # PE warm-up: ~18us of dummy matmuls so the HAM clock-gate releases
    # (1.2->2.4 GHz) while the first x/weight DMAs are still in flight.
    zsrc32 = wpool.tile([128, 512], f32, tag="warm32", bufs=1, name="zsrc32")
    nc.vector.memset(zsrc32[:], 0.0)
    zsrc = wpool.tile([128, 512], f32r, tag="warm", bufs=1, name="zsrc")
    nc.vector.tensor_copy(zsrc[:], zsrc32[:])
    pwarm = pspool.tile([128, 512], f32, tag="d0", bufs=2, name="pwarm")
    for i in range(56):
        nc.tensor.matmul(pwarm[:, :], zsrc[:, 0:128], zsrc[:, :],
                         start=(i == 0), stop=(i == 55))

    for b, (off, T) in enumerate(TBLOCKS):
        nch = T // 512                      # all chunks are exactly 512 wide
        chunks = [c0 * 512 for c0 in range(nch)]

        x_sb = xpool.tile([128, KD, T], f32r, tag="x", name=f"x{b}")
        w0 = {}
        if b == 0:
            # prefetch the m=0 weight tiles in engine-spread pieces before the
            # (larger) x loads so the first matmul isn't gated on one transfer
            for half in range(2):
                w_sb = wpool.tile([128, (KD // 2) * 256], f32r, tag="w",
                                  name=f"w{b}_0_{half}")
                wsl = gup[b, 0, :, half * (KD // 2) * 256:(half + 1) * (KD // 2) * 256]
                for j in range(KD // 2):
                    nc.sync.dma_start(w_sb[:, j * 256:(j + 1) * 256],
                                      wsl[:, j * 256:(j + 1) * 256])
                w0[half] = w_sb
            for ci in range(nch):
                nc.sync.dma_start(x_sb[:, 0, ci * 512:(ci + 1) * 512],
                                  xT[0:128, off + ci * 512:off + ci * 512 + 512])
            for k in range(1, KD):
                nc.sync.dma_start(x_sb[:, k, :], xT[k * 128:(k + 1) * 128, off:off + T])
        else:
            for k in range(KD):
                nc.sync.dma_start(x_sb[:, k, :], xT[k * 128:(k + 1) * 128, off:off + T])

        h_sb = hpool.tile([128, MF, T], f32r, tag="h", name=f"h{b}")

        # phase 1: gate/up matmuls + silu*up -> h
        for m in range(MF):
            pg = [pspool.tile([128, 512], f32, tag=f"g{ci}", name=f"pg{b}_{m}_{ci}")
                  for ci in range(nch)]
            pu = [pspool.tile([128, 512], f32, tag=f"u{ci}", name=f"pu{b}_{m}_{ci}")
                  for ci in range(nch)]
            for half in range(2):
                if b == 0 and m == 0:
                    w_sb = w0[half]
                else:
                    w_sb = wpool.tile([128, (KD // 2) * 256], f32r, tag="w",
                                      name=f"w{b}_{m}_{half}")
                    wsl = gup[b, m, :, half * (KD // 2) * 256:(half + 1) * (KD // 2) * 256]
                    hw = (KD // 2) * 128  # split across two DMA engines
                    nc.sync.dma_start(w_sb[:, :hw], wsl[:, :hw])
                    nc.sync.dma_start(w_sb[:, hw:], wsl[:, hw:])
                for j in range(KD // 2):
                    k = half * (KD // 2) + j
                    lg = w_sb[:, (2 * j) * 128:(2 * j + 1) * 128]
                    lu = w_sb[:, (2 * j + 1) * 128:(2 * j + 2) * 128]
                    for ci, c0 in enumerate(chunks):
                        nc.tensor.matmul(pg[ci][:, :], lg,
                                         x_sb[:, k, c0:c0 + 512],
                                         start=(k == 0), stop=(k == KD - 1))
                    for ci, c0 in enumerate(chunks):
                        nc.tensor.matmul(pu[ci][:, :], lu,
                                         x_sb[:, k, c0:c0 + 512],
                                         start=(k == 0), stop=(k == KD - 1))
            for ci, c0 in enumerate(chunks):
                sil = tpool.tile([128, 512], f32, tag="t", name=f"s{b}_{m}_{ci}")
                nc.scalar.activation(sil[:], pg[ci][:], silu)
                nc.vector.tensor_mul(h_sb[:, m, c0:c0 + 512], sil[:], pu[ci][:])

        # phase 2: down matmul -> yT
        for m2 in range(KD):
            wd_sb = wpool.tile([128, F], f32r, tag="wd", bufs=3, name=f"wd{b}_{m2}")
            nc.sync.dma_start(wd_sb[:, :F // 2], wdp[b, m2, :, :F // 2])
            nc.sync.dma_start(wd_sb[:, F // 2:], wdp[b, m2, :, F // 2:])
            pd = [pspool.tile([128, 512], f32, tag=f"d{ci}", bufs=2,
                              name=f"pd{b}_{m2}_{ci}") for ci in range(nch)]
            for kf in range(MF):
                ld = wd_sb[:, kf * 128:(kf + 1) * 128]
                for ci, c0 in enumerate(chunks):
                    nc.tensor.matmul(pd[ci][:, :], ld,
                                     h_sb[:, kf, c0:c0 + 512],
                                     start=(kf == 0), stop=(kf == MF - 1))
            for ci, c0 in enumerate(chunks):
                o_sb = opool.tile([128, 512], f32, tag="o", name=f"o{b}_{m2}_{ci}")
                nc.vector.tensor_copy(o_sb[:], pd[ci][:])
                nc.sync.dma_start(
                    yT[m2 * 128:(m2 + 1) * 128, off + c0:off + c0 + 512], o_sb[:])


def _build_program():
    from contextlib import ExitStack

    import concourse.tile as tile
    from concourse import bacc, mybir

    nc = bacc.Bacc("TRN2", target_bir_lowering=False, debug=False,
                   enable_asserts=False, num_devices=NCORES)
    f32 = mybir.dt.float32
    xT = nc.dram_tensor("xT", [D, TCOLS], mybir.dt.float32r, kind="ExternalInput").ap()
    gup = nc.dram_tensor("gup", [3, MF, 128, KD * 256], mybir.dt.float32r, kind="ExternalInput").ap()
    wdp = nc.dram_tensor("wdp", [3, KD, 128, F], mybir.dt.float32r, kind="ExternalInput").ap()
    yT = nc.dram_tensor("yT", [D, TCOLS], f32, kind="ExternalOutput").ap()

    with tile.TileContext(nc) as tc, ExitStack() as ctx:
        _emit(ctx, tc, xT, gup, wdp, yT)
    nc.compile()
    return nc


def _get_program():
    if "nc" not in _prog_cache:
        _prog_cache["nc"] = _build_program()
    return _prog_cache["nc"]


def _pack_gu(wg, wu):
    # [F, D] x2 -> [MF, 128, KD*2*128]; tile [:, :, (k*2+g)*128 + f]
    g = wg.reshape(MF, 128, KD, 128).transpose(0, 3, 2, 1)   # [m, p, k, f]
    u = wu.reshape(MF, 128, KD, 128).transpose(0, 3, 2, 1)
    return np.ascontiguousarray(
        np.stack([g, u], axis=3).reshape(MF, 128, KD * 256))


def _pack_wd(wd):
    # [D, F] -> [KD, 128, F]; tile [:, :, kf*128 + j] = wd[m2*128+j, kf*128+p]
    return np.ascontiguousarray(
        wd.reshape(KD, 128, MF, 128).transpose(0, 3, 2, 1).reshape(KD, 128, F))


def _swiglu_np(x, wg, wu, wd):
    # numpy fallback for capacity overflow (float32, matches reference math)
    a = x @ wg.T
    h = (a / (1.0 + np.exp(-a))) * (x @ wu.T)
    return h @ wd.T


def _ensure_axon_hooks():
    """Make ``antenv.axon_hooks`` importable (bass_utils needs it when
    BASS_TRACE=1 under axon; some images ship antenv without it)."""
    try:
        import antenv.axon_hooks  # noqa: F401
        return
    except ImportError:
        pass
    import sys
    import types

    mod = types.ModuleType("antenv.axon_hooks")
    mod._hook = None

    def set_axon_ntff_profile_hook(h):
        mod._hook = h

    def get_axon_ntff_profile_hook():
        return mod._hook

    mod.set_axon_ntff_profile_hook = set_axon_ntff_profile_hook
    mod.get_axon_ntff_profile_hook = get_axon_ntff_profile_hook
    try:
        import antenv

        sys.modules["antenv.axon_hooks"] = mod
        antenv.axon_hooks = mod
    except ImportError:
        return
    try:
        from trn_agent_boot.trn_boot import _ntff_profile_via_ctypes

        mod._hook = _ntff_profile_via_ctypes("/opt/axon/libaxon_pjrt.so")
    except Exception:
        pass


def kernel(x, gate_w, w_gate, w_up, w_down, sw_gate, sw_up, sw_down, expert_bias):
    from concourse.bass_utils import run_bass_kernel_spmd

    _ensure_axon_hooks()

    x = np.asarray(x, np.float32)
    gate_w = np.asarray(gate_w, np.float32)
    w_gate = np.asarray(w_gate, np.float32)
    w_up = np.asarray(w_up, np.float32)
    w_down = np.asarray(w_down, np.float32)
    sw_gate = np.asarray(sw_gate, np.float32)
    sw_up = np.asarray(sw_up, np.float32)
    sw_down = np.asarray(sw_down, np.float32)
    expert_bias = np.asarray(expert_bias, np.float32)

    flat = x.reshape(N, D)

    # ---- host gating / routing ----
    logits = flat.astype(np.float64) @ gate_w.astype(np.float64).T
    biased = logits + expert_bias.astype(np.float64)[None, :]
    order = np.argsort(-biased, axis=1, kind="stable")
    top_idx = order[:, :TOPK]                                  # [N, 2]
    m64 = logits.max(axis=1, keepdims=True)
    p = np.exp(logits - m64)
    probs = p / p.sum(axis=1, keepdims=True)
    top_w = np.take_along_axis(probs, top_idx, axis=1)
    top_w = top_w / top_w.sum(axis=1, keepdims=True)           # [N, 2]

    idx_list, w_list = [], []
    for e in range(E):
        sel = (top_idx == e)
        rows = np.where(sel.any(axis=1))[0]
        we = np.where(sel[rows, 0], top_w[rows, 0], top_w[rows, 1]).astype(np.float32)
        idx_list.append(rows)
        w_list.append(we)

    # ---- build per-core inputs ----
    sw_gu = _pack_gu(sw_gate, sw_up)
    sw_d = _pack_wd(sw_down)
    in_maps = []
    for c in range(NCORES):
        xT = np.zeros((D, TCOLS), np.float32)
        gu = np.empty((3, MF, 128, KD * 256), np.float32)
        wd = np.empty((3, KD, 128, F), np.float32)
        for half in range(2):
            e = 2 * c + half
            rows = idx_list[e][:C]
            xT[:, half * C:half * C + len(rows)] = flat[rows].T
            gu[half] = _pack_gu(w_gate[e], w_up[e])
            wd[half] = _pack_wd(w_down[e])
        xT[:, 2 * C:] = flat[c * NS:(c + 1) * NS].T
        gu[2] = sw_gu
        wd[2] = sw_d
        in_maps.append({"xT": xT, "gup": gu, "wdp": wd})

    # ---- run on 8 cores ----
    nc = _get_program()
    res = run_bass_kernel_spmd(nc, in_maps, core_ids=list(range(NCORES)))
    _prog_cache["last_results"] = res

    # ---- combine (expert-id order, then shared — matches reference) ----
    out = np.zeros((N, D), np.float32)
    for e in range(E):
        c, half = divmod(e, 2)
        rows, we = idx_list[e], w_list[e]
        ndev = min(len(rows), C)
        y = res.results[c]["yT"][:, half * C:half * C + ndev].T   # [ndev, D]
        out[rows[:ndev]] += we[:ndev, None] * y
        if len(rows) > C:  # capacity overflow: finish the tail on host
            r2 = rows[C:]
            y2 = _swiglu_np(flat[r2], w_gate[e], w_up[e], w_down[e])
            out[r2] += we[C:, None] * y2
    for c in range(NCORES):
        out[c * NS:(c + 1) * NS] += res.results[c]["yT"][:, 2 * C:].T

    return out.reshape(B, S, D)
